# revision 2
# baseline (speedup 1.0000x reference)
"""Trainium2 Bass kernel for CustomPositionsPiecewiseConv2d.

Math: for knots positions=[-1,-.5,0,.5,1] and inputs x in [0,1], the active
interpolation coefficients are
    c2 = relu(1-2v),  c4 = max(relu(2v-1), T),  c3 = 1 - c2 - c4
with T = 1[v >= theta] the isclose(v,1) mask.  Since c2+c3+c4 == 1 exactly
(everywhere, including the zero-padding border), the c3 plane folds away:
    out = C2 (x) (W2-W3) + C4 (x) (W4-W3) + sum_ck W3[o,c,k] + bias
Each plane is an elementwise function of v and v is shifted/padded copies of x,
so planes are computed once per padded image and the 3x3 im2col becomes 9
shifted access-pattern reads feeding PSUM-accumulated matmuls.

Modes:
  float32       exact, 4 cyc/row on PE
  float32r      tf32-rounded operands, 1 cyc/row (err ~1.6e-4)
  float32r_split  hi/lo tf32 split of coeffs+weights; per tap one K=128 matmul
                  [c2h,c4h,c2l,c4l]x[W2h,W4h,W2h,W4h] plus one K=64 matmul
                  [c2h,c4h]x[W2l,W4l]; drops only (lo x lo) terms ~2^-24.

Sharding: data-parallel over batch, 2 images per core on 8 cores.
"""

import numpy as np

B, C, H, W = 16, 32, 64, 64
O, P, KH, KW = 128, 5, 3, 3
NCORES = 8
IPC = B // NCORES            # images per core
HP, WP = H + 2, W + 2        # padded image (pad=1)
RT = 8                       # output rows per L-tile
NT = H // RT                 # L-tiles per image
K2 = KH * KW
L = H * W
ATOL = 1e-5
RTOL = 1e-5

MODE = "float32r"            # float32 | float32r | bf16_split


# ---------------------------------------------------------------- host math


def _isclose_np(a, b):
    return np.abs(a - b) <= np.float32(ATOL) + np.float32(RTOL) * np.abs(b)


def _reference_np(x, weights, bias, positions):
    """Direct numpy port of the reference (fallback path)."""
    EPS = 1e-6
    Bn, Cn, Hn, Wn = x.shape
    On, _, Pn, KHn, KWn = weights.shape
    xp = np.pad(x, ((0, 0), (0, 0), (1, 1), (1, 1)))
    cols = [
        xp[:, :, i : i + Hn, j : j + Wn] for i in range(KHn) for j in range(KWn)
    ]
    pat = np.stack(cols, axis=2)
    v = pat.reshape(Bn, Cn, KHn * KWn, Hn * Wn).astype(np.float32)

    left, right = positions[:-1], positions[1:]
    denom = right - left
    denom = np.where(denom == 0, np.float32(EPS), denom)
    varc = (1.0 / denom).astype(np.float32)
    const = (-left * varc).astype(np.float32)

    m_first = _isclose_np(v, positions[0])
    m_last = _isclose_np(v, positions[-1])
    in_range = (~(m_first | m_last)) & (v >= positions[0]) & (v <= positions[-1])

    coeff = np.zeros(v.shape + (Pn,), np.float32)
    coeff[..., 0] += m_first.astype(np.float32)
    coeff[..., Pn - 1] += m_last.astype(np.float32)
    for p in range(Pn - 1):
        m = (in_range & (v >= positions[p]) & (v < positions[p + 1])).astype(
            np.float32
        )
        t = v * varc[p] + const[p]
        coeff[..., p] += m * (1.0 - t)
        coeff[..., p + 1] += m * t

    Wk = np.transpose(weights, (0, 1, 3, 4, 2)).reshape(On, Cn, KHn * KWn, Pn)
    ident = np.all(np.abs(Wk - 1.0) <= np.float32(ATOL + RTOL), axis=-1)
    Wk_eff = np.where(ident[..., None], np.float32(0.0), Wk)

    out = np.einsum("bcklp,ockp->bol", coeff, Wk_eff, optimize=True)
    out = out + np.einsum(
        "bckl,ock->bol", v, ident.astype(np.float32), optimize=True
    )
    out = out + bias[None, :, None]
    return out.reshape(Bn, On, Hn, Wn).astype(np.float32)


def _compute_theta():
    """Smallest fp32 v such that fp32(1-v) <= fp32(ATOL + RTOL*1.0), matching
    the reference's m_last = isclose(v, 1.0) for v <= 1."""
    tau = np.float32(np.float32(ATOL) + np.float32(RTOL) * np.float32(1.0))
    th = np.float32(np.float32(1.0) - tau)
    while np.float32(np.float32(1.0) - np.nextafter(th, np.float32(0.0))) <= tau:
        th = np.nextafter(th, np.float32(0.0))
    while np.float32(np.float32(1.0) - th) > tau:
        th = np.nextafter(th, np.float32(2.0))
    return np.float32(th)


def _host_weights(weights, bias):
    """Fold c3 away.  Returns (wfold [2C, K2, O] f32 = [W2-W3; W4-W3],
    bias_eff [O] f32 = bias + sum_ck W3, ident_any)."""
    Wk = np.transpose(weights, (0, 1, 3, 4, 2)).reshape(O, C, K2, P)
    ident = np.all(np.abs(Wk - 1.0) <= np.float32(ATOL + RTOL), axis=-1)
    ident_any = bool(ident.any())
    Wk_eff = np.where(ident[..., None], np.float32(0.0), Wk)
    W2 = Wk_eff[:, :, :, 2].astype(np.float64)
    W3 = Wk_eff[:, :, :, 3].astype(np.float64)
    W4 = Wk_eff[:, :, :, 4].astype(np.float64)
    wfold = np.zeros((2 * C, K2, O), np.float32)
    wfold[0:C] = (W2 - W3).astype(np.float32).transpose(1, 2, 0)
    wfold[C : 2 * C] = (W4 - W3).astype(np.float32).transpose(1, 2, 0)
    bias_eff = (bias.astype(np.float64) + W3.sum(axis=(1, 2))).astype(np.float32)
    return np.ascontiguousarray(wfold), np.ascontiguousarray(bias_eff), ident_any


# ---------------------------------------------------------------- device IR


def _build_nc(theta, mode):
    import concourse.tile as tile
    from concourse import bacc, mybir

    f32 = mybir.dt.float32
    f32r = mybir.dt.float32r
    bf16 = mybir.dt.bfloat16
    Alu = mybir.AluOpType
    Act = mybir.ActivationFunctionType
    split = mode == "bf16_split"
    if mode == "float32":
        plane_dt = f32
    elif mode == "float32r":
        plane_dt = f32r
    else:
        plane_dt = bf16

    nc = bacc.Bacc("TRN2", target_bir_lowering=False, debug=False,
                   num_devices=NCORES)
    x_d = nc.dram_tensor("x", [IPC, C, H, W], f32, kind="ExternalInput").ap()
    w_d = nc.dram_tensor("wfold", [2 * C, K2, O], f32, kind="ExternalInput").ap()
    b_d = nc.dram_tensor("bias", [O, 1], f32, kind="ExternalInput").ap()
    o_d = nc.dram_tensor("out", [IPC, O, H, W], f32, kind="ExternalOutput").ap()

    with tile.TileContext(nc) as tc:
        with (
            tc.tile_pool(name="const", bufs=1) as constp,
            tc.tile_pool(name="scratch", bufs=1) as scrp,
            tc.tile_pool(name="plane", bufs=1) as planep,
            tc.tile_pool(name="ybuf", bufs=2) as ybufp,
            tc.tile_pool(name="psum", bufs=1, space="PSUM") as psump,
            tc.tile_pool(name="osb", bufs=4) as osbp,
        ):
            # ---- x loads first (phi critical path), weights after ----
            XF = scrp.tile([IPC * C, H, W], f32)      # flat x
            for i in range(IPC):
                nc.sync.dma_start(XF[i * C : (i + 1) * C], x_d[i])

            # pull the ACT table load off the critical path
            tiny = constp.tile([C, 1], f32)
            nc.gpsimd.memset(tiny[:], 0.0)
            nc.scalar.activation(tiny[:], tiny[:], Act.Relu, bias=0.0, scale=1.0)

            # PE warmup: dummy matmuls keep HAM at K=8/8 until the real
            # stream starts (otherwise the first ~5us of matmuls run at 1.2GHz)
            zb = constp.tile([128, 512], plane_dt)
            nc.gpsimd.memset(
                zb[:].bitcast(f32) if plane_dt == f32r else zb[:], 0.0
            )
            warm_ctr = [0]

            def warm(nmm, rhs=None):
                """Dummy matmuls (results never read). rhs gates when the
                batch can start, chaining PE busy-ness across the phi phase."""
                w = warm_ctr[0]
                warm_ctr[0] += 1
                pw = psump.tile(
                    [O, 512], f32, name=f"ps_warm{w}", tag=f"ps{w % 2}"
                )
                r = zb[:] if rhs is None else rhs
                kp = r.shape[0]
                for j in range(nmm):
                    nc.tensor.matmul(
                        pw[:], zb[0:kp, 0:128], r,
                        start=(j % 8 == 0), stop=(j % 8 == 7 or j == nmm - 1),
                    )

            warm(16)

            # ---- weights ----
            w_sb = constp.tile([2 * C, K2, O], f32)
            nc.sync.dma_start(w_sb[:], w_d[:])
            b_sb = constp.tile([O, 1], f32)
            nc.sync.dma_start(b_sb[:], b_d[:])
            if mode == "float32":
                w_hi = w_sb
            else:
                w_hi = constp.tile([2 * C, K2, O], plane_dt)
                nc.vector.tensor_copy(w_hi[:], w_sb[:])
            if split:
                w_lo = constp.tile([2 * C, K2, O], plane_dt)
                nc.vector.tensor_tensor(w_lo[:], w_sb[:], w_hi[:], Alu.subtract)
                # lhsT1 rows: [W2h, W4h, W2h, W4h] (hi coeffs then lo coeffs)
                w_rep = constp.tile([4 * C, K2, O], plane_dt)
                nc.sync.dma_start(w_rep[0 : 2 * C], w_hi[:])
                nc.sync.dma_start(w_rep[2 * C : 4 * C], w_hi[:])
                lhs1, lhs2 = w_rep, w_lo
            else:
                lhs1, lhs2 = w_hi, None

            # ---- coefficient planes ----
            # scratch on the same partitions as each image's plane slice
            # (engine ops require equal SBUF base partitions across operands)
            RF = scrp.tile([IPC * C, H, W], f32)
            CF = scrp.tile([IPC * C, H, W], f32)

            npl = 4 if split else 2
            # plane buffers, padded layout; group order:
            #   split: [c2h, c4h, c2l, c4l]   else: [c2, c4]
            PL = [
                planep.tile([IPC * C, HP, WP], plane_dt, name=f"PL{g}")
                for g in range(npl)
            ]
            # borders: c2-like planes = 1 at v=0, everything else = 0
            # (memset rejects f32r dests; same-size bitcast to f32 is a no-op)
            for g, pl in enumerate(PL):
                bv = 1.0 if g == 0 else 0.0
                for strip in (
                    pl[:, 0, :],
                    pl[:, HP - 1, :],
                    pl[:, 1 : HP - 1, 0],
                    pl[:, 1 : HP - 1, WP - 1],
                ):
                    nc.gpsimd.memset(
                        strip.bitcast(f32) if plane_dt == f32r else strip, bv
                    )

            def interior(pl):
                return pl[:, 1 : HP - 1, 1 : WP - 1]

            negone = constp.tile([IPC * C, 1], f32)
            nc.gpsimd.memset(negone[:], -1.0)

            def phi_chunk(r0, r1):
                """Coefficient planes for image rows [r0, r1), both images at
                once (64 partitions). Chunking lets the first GEMM tiles start
                while the rest of the planes are still being computed."""
                xf = XF[:, r0:r1, :]
                neg = negone[:]
                rf = RF[:, r0:r1, :]
                cf = CF[:, r0:r1, :]
                pls = [pl[:, 1 + r0 : 1 + r1, 1 : WP - 1] for pl in PL]
                if split:
                    # bf16 rounding absorbs the isclose(v,1) mask: for
                    # v >= 1-2^-9, relu(2v-1) rounds to exactly 1.0, and the
                    # lo-plane residual lands on hi weights scaled 2^-9.
                    c2h, c4h, c2l, c4l = pls
                    nc.scalar.activation(rf, xf, Act.Relu, bias=neg, scale=2.0)
                    nc.vector.tensor_copy(c4h, rf)
                    nc.vector.tensor_tensor(c4l, rf, c4h, Alu.subtract)
                    nc.scalar.activation(cf, xf, Act.Relu, bias=1.0, scale=-2.0)
                    nc.scalar.activation(c2h, cf, Act.Copy)
                    nc.vector.tensor_tensor(c2l, cf, c2h, Alu.subtract)
                else:
                    c2, c4 = pls
                    nc.vector.tensor_scalar(cf, xf, float(theta), None, Alu.is_ge)
                    nc.scalar.activation(rf, xf, Act.Relu, bias=neg, scale=2.0)
                    nc.vector.tensor_tensor(rf, rf, cf, Alu.max)
                    nc.vector.tensor_copy(c4, rf)
                    nc.scalar.activation(c2, xf, Act.Relu, bias=1.0, scale=-2.0)

            phi_chunk(0, H)
            # bridge PE busy-ness across the phi phase: each batch is gated
            # on a progressively later plane artifact (HAM re-throttles after
            # ~3.4us of PE idle, and a cold stream runs at 1.2GHz)
            nh = npl * C // 2
            order = [1, 3, 0, 2] if split else [1, 0]
            for g in order:
                warm(8, PL[g][0:nh, 0:RT, 0:W])

            # ---- per-image gather + GEMM ----
            # Tap-outer loop: one LDWEIGHTS feeds a run of back-to-back
            # matmuls (same stationary operand), so drain overlaps the next
            # fill and the per-MM cost stays ~N/2.4 instead of the isolated
            # latency. All 8 L-tiles of an image accumulate in 8 PSUM banks.
            # Tiles are issued in two groups gated on the two phi row-chunks
            # (Tile deps are byte-range granular), so the GEMM starts as soon
            # as the first chunk of planes is gathered into Y.
            def tap_loop(Y, pss, tiles, start, stop):
                for ki in range(K2):
                    kh, kw = divmod(ki, KW)
                    cols = slice(kw, kw + W)
                    last = ki == K2 - 1
                    for t in tiles:
                        rows = slice(t * RT + kh, t * RT + kh + RT)
                        nc.tensor.matmul(
                            pss[t][:], lhs1[:, ki, :], Y[:, rows, cols],
                            start=(start and ki == 0),
                            stop=(stop and last and not split),
                        )
                    if split:
                        for t in tiles:
                            rows = slice(t * RT + kh, t * RT + kh + RT)
                            nc.tensor.matmul(
                                pss[t][:], lhs2[:, ki, :],
                                Y[0 : 2 * C, rows, cols],
                                start=False, stop=(stop and last),
                            )

            for i in range(IPC):
                Y = ybufp.tile([npl * C, HP, WP], plane_dt, name="Y", tag="Y")
                s = slice(i * C, (i + 1) * C)
                for g, pl in enumerate(PL):
                    nc.sync.dma_start(Y[g * C : (g + 1) * C], pl[s])

                pss = [
                    psump.tile([O, RT * W], f32, name=f"ps{t}", tag=f"ps{t}")
                    for t in range(NT)
                ]
                tap_loop(Y, pss, list(range(NT)), start=True, stop=True)
                for t in range(NT):
                    osb = osbp.tile([O, RT * W], f32, name="osb")
                    if t % 2 == 0:
                        nc.scalar.activation(
                            osb[:], pss[t][:], Act.Identity, bias=b_sb[:, 0:1],
                            scale=1.0,
                        )
                    else:
                        nc.vector.tensor_scalar(
                            osb[:], pss[t][:], b_sb[:, 0:1], None, Alu.add
                        )
                    nc.sync.dma_start(
                        o_d[i, :, t * RT : (t + 1) * RT, :],
                        osb[:].rearrange("o (r w) -> o r w", r=RT),
                    )
    nc.compile()
    return nc


# ---------------------------------------------------------------- entry


def _prep(inputs):
    x = np.ascontiguousarray(np.asarray(inputs["x"], dtype=np.float32))
    weights = np.ascontiguousarray(np.asarray(inputs["weights"], dtype=np.float32))
    bias = np.ascontiguousarray(np.asarray(inputs["bias"], dtype=np.float32))
    positions = np.ascontiguousarray(
        np.asarray(inputs["positions"], dtype=np.float32)
    )
    return x, weights, bias, positions


def _fast_path_ok(x, positions):
    expect = np.linspace(-1.0, 1.0, P, dtype=np.float32)
    return (
        x.shape == (B, C, H, W)
        and positions.shape == (P,)
        and np.array_equal(positions, expect)
        and float(x.min()) >= 0.0
        and float(x.max()) <= 1.0
    )


def kernel(**inputs):
    x, weights, bias, positions = _prep(inputs)
    if not _fast_path_ok(x, positions):
        return _reference_np(x, weights, bias, positions)

    wfold, bias_eff, ident_any = _host_weights(weights, bias)
    if ident_any:
        # identity-shortcut weights present: needs the raw-v plane; use the
        # exact fallback rather than a rarely-exercised device path
        return _reference_np(x, weights, bias, positions)

    from concourse.bass_utils import run_bass_kernel_spmd

    nc = _build_nc(_compute_theta(), MODE)
    bias2d = np.ascontiguousarray(bias_eff.reshape(O, 1))
    in_maps = [
        {"x": np.ascontiguousarray(x[i * IPC : (i + 1) * IPC]),
         "wfold": wfold, "bias": bias2d}
        for i in range(NCORES)
    ]
    res = run_bass_kernel_spmd(nc, in_maps, core_ids=list(range(NCORES)))
    out = np.concatenate([res.results[i]["out"] for i in range(NCORES)], axis=0)
    return np.ascontiguousarray(out)


# ------------------------------------------------------------ dev utilities


def _run_sim(inputs):
    """CoreSim single-core run (images 0..IPC-1) for correctness debugging."""
    from concourse.bass_interp import CoreSim

    x, weights, bias, positions = _prep(inputs)
    assert _fast_path_ok(x, positions)
    wfold, bias_eff, ident_any = _host_weights(weights, bias)
    assert not ident_any
    nc = _build_nc(_compute_theta(), MODE)
    sim = CoreSim(nc)
    sim.tensor("x")[:] = x[:IPC]
    sim.tensor("wfold")[:] = wfold
    sim.tensor("bias")[:] = bias_eff.reshape(O, 1)
    sim.simulate()
    return np.array(sim.tensor("out"))



# revision 4
# speedup vs baseline: 1.1781x; 1.1781x over previous
"""Trainium2 Bass kernel for CustomPositionsPiecewiseConv2d.

Math: for knots positions=[-1,-.5,0,.5,1] and x in [0,1], the active
interpolation coefficients are c2 = relu(1-2v), c4 = relu(2v-1),
c3 = 1 - c2 - c4 (exactly, everywhere incl. the zero-padding border), so
    out = C2 (x) (W2-W3) + C4 (x) (W4-W3) + sum_ck W3[o,c,k] + bias
Each plane is elementwise in v; the 3x3 im2col becomes shifted access-pattern
reads feeding PSUM-accumulated matmuls.  bf16 rounding absorbs the
isclose(v,1) mask (relu(2v-1) rounds to exactly 1.0 there); total rel err
~1e-3 vs the 2e-2 gate.

Layouts:
  PLB [2C, 2, HP, WP] bf16 - plane index is a FREE dim, so one DMA gathers
  the per-image, channel-interleaved Y = [c2/c4 x 32ch] the GEMM wants; the
  weights are row-interleaved on host to match (row 2c+g = plane-g, chan c).
  PAIR mode adds a second Y half = planes shifted one column, pairing taps
  (r,0)+(r,1) into K=128 matmuls; taps (r,2) read the shifted half at +1 col
  as K=64 singles -> 6 passes/tile instead of 9.

Pipeline: x load, phi (coeff planes), Y gather, GEMM, PSUM drain and out DMA
are all chunked into 4 row-bands per image and software-pipelined, so the PE
streams matmuls continuously from ~4us on and the tail after the last matmul
is one group's drain.

Sharding: data-parallel over batch, 2 images per core on 8 cores.
"""

import numpy as np

B, C, H, W = 16, 32, 64, 64
O, P, KH, KW = 128, 5, 3, 3
NCORES = 8
IPC = B // NCORES            # images per core
HP, WP = H + 2, W + 2        # padded image (pad=1)
RT = 8                       # output rows per PSUM tile
NT = H // RT                 # PSUM tiles per image
GR = 2                       # tiles per drain group
NG = NT // GR                # groups per image
K2 = KH * KW
ATOL = 1e-5
RTOL = 1e-5

PAIR = False                 # pair taps (r,0)+(r,1) into K=128 matmuls
WARM = 10                    # PE warmup matmuls (clock ramp)

# phi/gather row chunks (padded-row bands, group g needs bands 0..g)
BANDS = [(0, 18), (18, 34), (34, 50), (50, 66)]


# ---------------------------------------------------------------- host math


def _isclose_np(a, b):
    return np.abs(a - b) <= np.float32(ATOL) + np.float32(RTOL) * np.abs(b)


def _reference_np(x, weights, bias, positions):
    """Direct numpy port of the reference (fallback path)."""
    EPS = 1e-6
    Bn, Cn, Hn, Wn = x.shape
    On, _, Pn, KHn, KWn = weights.shape
    xp = np.pad(x, ((0, 0), (0, 0), (1, 1), (1, 1)))
    cols = [
        xp[:, :, i : i + Hn, j : j + Wn] for i in range(KHn) for j in range(KWn)
    ]
    pat = np.stack(cols, axis=2)
    v = pat.reshape(Bn, Cn, KHn * KWn, Hn * Wn).astype(np.float32)

    left, right = positions[:-1], positions[1:]
    denom = right - left
    denom = np.where(denom == 0, np.float32(EPS), denom)
    varc = (1.0 / denom).astype(np.float32)
    const = (-left * varc).astype(np.float32)

    m_first = _isclose_np(v, positions[0])
    m_last = _isclose_np(v, positions[-1])
    in_range = (~(m_first | m_last)) & (v >= positions[0]) & (v <= positions[-1])

    coeff = np.zeros(v.shape + (Pn,), np.float32)
    coeff[..., 0] += m_first.astype(np.float32)
    coeff[..., Pn - 1] += m_last.astype(np.float32)
    for p in range(Pn - 1):
        m = (in_range & (v >= positions[p]) & (v < positions[p + 1])).astype(
            np.float32
        )
        t = v * varc[p] + const[p]
        coeff[..., p] += m * (1.0 - t)
        coeff[..., p + 1] += m * t

    Wk = np.transpose(weights, (0, 1, 3, 4, 2)).reshape(On, Cn, KHn * KWn, Pn)
    ident = np.all(np.abs(Wk - 1.0) <= np.float32(ATOL + RTOL), axis=-1)
    Wk_eff = np.where(ident[..., None], np.float32(0.0), Wk)

    out = np.einsum("bcklp,ockp->bol", coeff, Wk_eff, optimize=True)
    out = out + np.einsum(
        "bckl,ock->bol", v, ident.astype(np.float32), optimize=True
    )
    out = out + bias[None, :, None]
    return out.reshape(Bn, On, Hn, Wn).astype(np.float32)


def _host_weights(weights, bias):
    """Fold c3 away and interleave rows to match the device plane layout.

    Returns (winter [2C, K2, O] f32 with row 2c+g = (W{2,4}-W3)[:,c,k].T,
    bias_eff [O] f32 = bias + sum_ck W3, ident_any)."""
    Wk = np.transpose(weights, (0, 1, 3, 4, 2)).reshape(O, C, K2, P)
    ident = np.all(np.abs(Wk - 1.0) <= np.float32(ATOL + RTOL), axis=-1)
    ident_any = bool(ident.any())
    Wk_eff = np.where(ident[..., None], np.float32(0.0), Wk)
    W3 = Wk_eff[:, :, :, 3].astype(np.float64)
    W2 = Wk_eff[:, :, :, 2].astype(np.float64) - W3   # c2 weights [O,C,K2]
    W4 = Wk_eff[:, :, :, 4].astype(np.float64) - W3   # c4 weights
    winter = np.zeros((2 * C, K2, O), np.float32)
    winter[0::2] = W2.astype(np.float32).transpose(1, 2, 0)
    winter[1::2] = W4.astype(np.float32).transpose(1, 2, 0)
    bias_eff = (bias.astype(np.float64) + W3.sum(axis=(1, 2))).astype(np.float32)
    return winter, np.ascontiguousarray(bias_eff), ident_any


def _pack_weights(winter):
    """Device weight tensors (bf16) for the chosen tap schedule."""
    import ml_dtypes

    bf = ml_dtypes.bfloat16
    if not PAIR:
        return {"wint": np.ascontiguousarray(winter.astype(bf))}
    # pair pass r: lo rows = tap (r,0), hi rows = tap (r,1);
    # single pass r: tap (r,2) read from the hi (shifted) Y half
    wpair = np.zeros((4 * C, KH, O), np.float32)
    wsing = np.zeros((2 * C, KH, O), np.float32)
    for r in range(KH):
        wpair[0 : 2 * C, r] = winter[:, r * KW + 0]
        wpair[2 * C : 4 * C, r] = winter[:, r * KW + 1]
        wsing[:, r] = winter[:, r * KW + 2]
    return {
        "wpair": np.ascontiguousarray(wpair.astype(bf)),
        "wsing": np.ascontiguousarray(wsing.astype(bf)),
    }


# ---------------------------------------------------------------- device IR


def _build_nc():
    import concourse.tile as tile
    from concourse import bacc, mybir

    f32 = mybir.dt.float32
    bf16 = mybir.dt.bfloat16
    Alu = mybir.AluOpType
    Act = mybir.ActivationFunctionType

    nc = bacc.Bacc("TRN2", target_bir_lowering=False, debug=False,
                   num_devices=NCORES)
    x_d = nc.dram_tensor("x", [IPC, C, H, W], f32, kind="ExternalInput").ap()
    if PAIR:
        wp_d = nc.dram_tensor("wpair", [4 * C, KH, O], bf16,
                              kind="ExternalInput").ap()
        ws_d = nc.dram_tensor("wsing", [2 * C, KH, O], bf16,
                              kind="ExternalInput").ap()
    else:
        wi_d = nc.dram_tensor("wint", [2 * C, K2, O], bf16,
                              kind="ExternalInput").ap()
    b_d = nc.dram_tensor("bias", [O, 1], f32, kind="ExternalInput").ap()
    o_d = nc.dram_tensor("out", [IPC, O, H, W], f32, kind="ExternalOutput").ap()

    YPART = 4 * C if PAIR else 2 * C

    with tile.TileContext(nc) as tc:
        with (
            tc.tile_pool(name="const", bufs=1) as constp,
            tc.tile_pool(name="scratch", bufs=1) as scrp,
            tc.tile_pool(name="ybuf", bufs=2) as ybufp,
            tc.tile_pool(name="psum", bufs=1, space="PSUM") as psump,
            tc.tile_pool(name="osb", bufs=2) as osbp,
        ):
            XF = scrp.tile([IPC * C, H, W], f32)
            # x row-band 0 for both images first (phi critical path)
            for i in range(IPC):
                nc.sync.dma_start(XF[i * C : (i + 1) * C, 0:17], x_d[i, :, 0:17])

            # ACT table preload off the critical path
            tiny = constp.tile([IPC * C, 1], f32)
            nc.gpsimd.memset(tiny[:], 0.0)
            nc.scalar.activation(tiny[:], tiny[:], Act.Relu, bias=0.0, scale=1.0)

            # PE clock ramp: dummy matmuls (results never read); use the last
            # group's PSUM banks so group 0 isn't blocked on the warm drain.
            zb = constp.tile([128, 512], bf16)
            nc.gpsimd.memset(zb[:], 0.0)
            pw = [psump.tile([O, 512], f32, name=f"ps_warm{k}",
                             tag=f"ps{6 + k}") for k in range(2)]
            for j in range(WARM):
                nc.tensor.matmul(pw[j % 2][:], zb[0:128, 0:128], zb[:],
                                 start=(j < 2), stop=(j >= WARM - 2))

            # weights + bias + rest of x
            if PAIR:
                wp_sb = constp.tile([4 * C, KH, O], bf16)
                nc.sync.dma_start(wp_sb[:], wp_d[:])
                ws_sb = constp.tile([2 * C, KH, O], bf16)
                nc.sync.dma_start(ws_sb[:], ws_d[:])
            else:
                wi_sb = constp.tile([2 * C, K2, O], bf16)
                nc.sync.dma_start(wi_sb[:], wi_d[:])
            b_sb = constp.tile([O, 1], f32)
            nc.sync.dma_start(b_sb[:], b_d[:])
            for i in range(IPC):
                nc.sync.dma_start(XF[i * C : (i + 1) * C, 17:H],
                                  x_d[i, :, 17:H])

            # coefficient planes, plane index as free dim: [2C, {c2,c4}, HP, WP]
            PLB = scrp.tile([IPC * C, 2, HP, WP], bf16)
            for g in range(2):
                bv = 1.0 if g == 0 else 0.0
                for strip in (
                    PLB[:, g, 0, :],
                    PLB[:, g, HP - 1, :],
                    PLB[:, g, 1 : HP - 1, 0],
                    PLB[:, g, 1 : HP - 1, WP - 1],
                ):
                    nc.gpsimd.memset(strip, bv)

            negone = constp.tile([IPC * C, 1], f32)
            nc.gpsimd.memset(negone[:], -1.0)

            def phi_band(pr0, pr1):
                """c2/c4 planes for padded rows [pr0,pr1) (both images)."""
                r0, r1 = max(pr0, 1) - 1, min(pr1, HP - 1) - 1
                xf = XF[:, r0:r1]
                nc.scalar.activation(PLB[:, 1, 1 + r0 : 1 + r1, 1 : WP - 1],
                                     xf, Act.Relu, bias=negone[:], scale=2.0)
                nc.scalar.activation(PLB[:, 0, 1 + r0 : 1 + r1, 1 : WP - 1],
                                     xf, Act.Relu, bias=1.0, scale=-2.0)

            for pr0, pr1 in BANDS:
                phi_band(pr0, pr1)

            def gather(Y, i, pr0, pr1):
                """One DMA: Y[0:2C, band] = channel-interleaved c2/c4 of
                image i; PAIR adds the col+1-shifted copy in the hi half."""
                src = PLB[i * C : (i + 1) * C, :, pr0:pr1]
                nc.sync.dma_start(Y[0 : 2 * C, pr0:pr1], src)
                if PAIR:
                    nc.sync.dma_start(
                        Y[2 * C : 4 * C, pr0:pr1, 0 : WP - 1],
                        PLB[i * C : (i + 1) * C, :, pr0:pr1, 1:WP],
                    )

            def mm_tile(Y, ps, t, first, last):
                """All tap passes for PSUM tile t (output rows RT*t..+RT)."""
                if PAIR:
                    for r in range(KH):
                        rows = slice(t * RT + r, t * RT + r + RT)
                        nc.tensor.matmul(ps[:], wp_sb[:, r, :],
                                         Y[:, rows, 0:W],
                                         start=(first and r == 0), stop=False)
                    for r in range(KH):
                        rows = slice(t * RT + r, t * RT + r + RT)
                        nc.tensor.matmul(ps[:], ws_sb[:, r, :],
                                         Y[2 * C : 4 * C, rows, 1 : W + 1],
                                         start=False,
                                         stop=(last and r == KH - 1))
                else:
                    for ki in range(K2):
                        kh, kw = divmod(ki, KW)
                        rows = slice(t * RT + kh, t * RT + kh + RT)
                        nc.tensor.matmul(ps[:], wi_sb[:, ki, :],
                                         Y[:, rows, kw : kw + W],
                                         start=(first and ki == 0),
                                         stop=(last and ki == K2 - 1))

            for i in range(IPC):
                Y = ybufp.tile([YPART, HP, WP], bf16, name="Y", tag="Y")
                for pr0, pr1 in BANDS:
                    gather(Y, i, pr0, pr1)
                for g in range(NG):
                    osb = osbp.tile([O, GR, RT * W], f32, name="osb")
                    for j in range(GR):
                        t = g * GR + j
                        ps = psump.tile([O, RT * W], f32, name=f"ps{t}",
                                        tag=f"ps{t}")
                        mm_tile(Y, ps, t, first=True, last=True)
                        if t % 2 == 0:
                            nc.scalar.activation(osb[:, j], ps[:],
                                                 Act.Identity,
                                                 bias=b_sb[:, 0:1], scale=1.0)
                        else:
                            nc.vector.tensor_scalar(osb[:, j], ps[:],
                                                    b_sb[:, 0:1], None, Alu.add)
                    nc.sync.dma_start(
                        o_d[i, :, g * GR * RT : (g + 1) * GR * RT, :],
                        osb[:].rearrange("o g (r w) -> o (g r) w", r=RT),
                    )
    nc.compile()
    return nc


# ---------------------------------------------------------------- entry


def _prep(inputs):
    x = np.ascontiguousarray(np.asarray(inputs["x"], dtype=np.float32))
    weights = np.ascontiguousarray(np.asarray(inputs["weights"], dtype=np.float32))
    bias = np.ascontiguousarray(np.asarray(inputs["bias"], dtype=np.float32))
    positions = np.ascontiguousarray(
        np.asarray(inputs["positions"], dtype=np.float32)
    )
    return x, weights, bias, positions


def _fast_path_ok(x, positions):
    expect = np.linspace(-1.0, 1.0, P, dtype=np.float32)
    return (
        x.shape == (B, C, H, W)
        and positions.shape == (P,)
        and np.array_equal(positions, expect)
        and float(x.min()) >= 0.0
        and float(x.max()) <= 1.0
    )


def kernel(**inputs):
    x, weights, bias, positions = _prep(inputs)
    if not _fast_path_ok(x, positions):
        return _reference_np(x, weights, bias, positions)

    winter, bias_eff, ident_any = _host_weights(weights, bias)
    if ident_any:
        # identity-shortcut weights present: needs the raw-v plane; use the
        # exact fallback rather than a rarely-exercised device path
        return _reference_np(x, weights, bias, positions)

    from concourse.bass_utils import run_bass_kernel_spmd

    nc = _build_nc()
    wmap = _pack_weights(winter)
    bias2d = np.ascontiguousarray(bias_eff.reshape(O, 1))
    in_maps = [
        {"x": np.ascontiguousarray(x[i * IPC : (i + 1) * IPC]),
         "bias": bias2d, **wmap}
        for i in range(NCORES)
    ]
    res = run_bass_kernel_spmd(nc, in_maps, core_ids=list(range(NCORES)))
    out = np.concatenate([res.results[i]["out"] for i in range(NCORES)], axis=0)
    return np.ascontiguousarray(out.astype(np.float32))


# ------------------------------------------------------------ dev utilities


def _run_sim(inputs):
    """CoreSim single-core run (images 0..IPC-1) for correctness debugging."""
    from concourse.bass_interp import CoreSim

    x, weights, bias, positions = _prep(inputs)
    assert _fast_path_ok(x, positions)
    winter, bias_eff, ident_any = _host_weights(weights, bias)
    assert not ident_any
    nc = _build_nc()
    sim = CoreSim(nc)
    sim.tensor("x")[:] = x[:IPC]
    for k, v in _pack_weights(winter).items():
        sim.tensor(k)[:] = v
    sim.tensor("bias")[:] = bias_eff.reshape(O, 1)
    sim.simulate()
    return np.array(sim.tensor("out"))


# revision 13
# speedup vs baseline: 1.7711x; 1.5033x over previous
"""Trainium2 Bass kernel for CustomPositionsPiecewiseConv2d.

Math: for knots positions=[-1,-.5,0,.5,1] and x in [0,1], the active
interpolation coefficients are c2 = relu(1-2v), c4 = relu(2v-1),
c3 = 1 - c2 - c4 (exactly, everywhere incl. the zero-padding border), so
    out = C2 (x) (W2-W3) + C4 (x) (W4-W3) + sum_ck W3[o,c,k] + bias
Each plane is elementwise in v; the 3x3 im2col becomes shifted access-pattern
reads feeding PSUM-accumulated matmuls.  bf16 rounding absorbs the
isclose(v,1) mask (relu(2v-1) rounds to exactly 1.0 there); total rel err
~1e-3 vs the 2e-2 gate.

Layouts:
  PLB [2C, 2, HP, WP] bf16 - plane index is a FREE dim, so one DMA gathers
  the per-image, channel-interleaved Y = [c2/c4 x 32ch] the GEMM wants; the
  weights are row-interleaved on host to match (row 2c+g = plane-g, chan c).
  PAIR mode adds a second Y half = planes shifted one column, pairing taps
  (r,0)+(r,1) into K=128 matmuls; taps (r,2) read the shifted half at +1 col
  as K=64 singles -> 6 passes/tile instead of 9.

Pipeline: x load, phi (coeff planes), Y gather, GEMM, PSUM drain and out DMA
are all chunked into 4 row-bands per image and software-pipelined, so the PE
streams matmuls continuously from ~4us on and the tail after the last matmul
is one group's drain.

Sharding: data-parallel over batch, 2 images per core on 8 cores.
"""

import numpy as np

B, C, H, W = 16, 32, 64, 64
O, P, KH, KW = 128, 5, 3, 3
NCORES = 8
IPC = B // NCORES            # images per core
HP, WP = H + 2, W + 2        # padded image (pad=1)
RT = 8                       # output rows per PSUM tile
NT = H // RT                 # PSUM tiles per image
GR = 2                       # tiles per drain group
NG = NT // GR                # groups per image
K2 = KH * KW
ATOL = 1e-5
RTOL = 1e-5

PAIR = True                  # pair taps (r,0)+(r,1) into K=128 matmuls
WARM = 10                    # PE warmup matmuls (clock ramp)

# phi/gather row chunks (padded-row bands, group g needs bands 0..g)
BANDS = [(0, 18), (18, 34), (34, 50), (50, 66)]


# ---------------------------------------------------------------- host math


def _isclose_np(a, b):
    return np.abs(a - b) <= np.float32(ATOL) + np.float32(RTOL) * np.abs(b)


def _reference_np(x, weights, bias, positions):
    """Direct numpy port of the reference (fallback path)."""
    EPS = 1e-6
    Bn, Cn, Hn, Wn = x.shape
    On, _, Pn, KHn, KWn = weights.shape
    xp = np.pad(x, ((0, 0), (0, 0), (1, 1), (1, 1)))
    cols = [
        xp[:, :, i : i + Hn, j : j + Wn] for i in range(KHn) for j in range(KWn)
    ]
    pat = np.stack(cols, axis=2)
    v = pat.reshape(Bn, Cn, KHn * KWn, Hn * Wn).astype(np.float32)

    left, right = positions[:-1], positions[1:]
    denom = right - left
    denom = np.where(denom == 0, np.float32(EPS), denom)
    varc = (1.0 / denom).astype(np.float32)
    const = (-left * varc).astype(np.float32)

    m_first = _isclose_np(v, positions[0])
    m_last = _isclose_np(v, positions[-1])
    in_range = (~(m_first | m_last)) & (v >= positions[0]) & (v <= positions[-1])

    coeff = np.zeros(v.shape + (Pn,), np.float32)
    coeff[..., 0] += m_first.astype(np.float32)
    coeff[..., Pn - 1] += m_last.astype(np.float32)
    for p in range(Pn - 1):
        m = (in_range & (v >= positions[p]) & (v < positions[p + 1])).astype(
            np.float32
        )
        t = v * varc[p] + const[p]
        coeff[..., p] += m * (1.0 - t)
        coeff[..., p + 1] += m * t

    Wk = np.transpose(weights, (0, 1, 3, 4, 2)).reshape(On, Cn, KHn * KWn, Pn)
    ident = np.all(np.abs(Wk - 1.0) <= np.float32(ATOL + RTOL), axis=-1)
    Wk_eff = np.where(ident[..., None], np.float32(0.0), Wk)

    out = np.einsum("bcklp,ockp->bol", coeff, Wk_eff, optimize=True)
    out = out + np.einsum(
        "bckl,ock->bol", v, ident.astype(np.float32), optimize=True
    )
    out = out + bias[None, :, None]
    return out.reshape(Bn, On, Hn, Wn).astype(np.float32)


def _host_weights(weights, bias):
    """Fold c3 away and interleave rows to match the device plane layout.

    Returns (winter [2C, K2, O] f32 with row 2c+g = (W{2,4}-W3)[:,c,k].T,
    bias_eff [O] f32 = bias + sum_ck W3, ident_any)."""
    Wk = np.transpose(weights, (0, 1, 3, 4, 2)).reshape(O, C, K2, P)
    ident = np.all(np.abs(Wk - 1.0) <= np.float32(ATOL + RTOL), axis=-1)
    ident_any = bool(ident.any())
    Wk_eff = np.where(ident[..., None], np.float32(0.0), Wk)
    W3 = Wk_eff[:, :, :, 3].astype(np.float64)
    W2 = Wk_eff[:, :, :, 2].astype(np.float64) - W3   # c2 weights [O,C,K2]
    W4 = Wk_eff[:, :, :, 4].astype(np.float64) - W3   # c4 weights
    winter = np.zeros((2 * C, K2, O), np.float32)
    winter[0::2] = W2.astype(np.float32).transpose(1, 2, 0)
    winter[1::2] = W4.astype(np.float32).transpose(1, 2, 0)
    bias_eff = (bias.astype(np.float64) + W3.sum(axis=(1, 2))).astype(np.float32)
    return winter, np.ascontiguousarray(bias_eff), ident_any


def _pack_weights(winter):
    """Device weight tensors (bf16) for the chosen tap schedule."""
    import ml_dtypes

    bf = ml_dtypes.bfloat16
    if not PAIR:
        return {"wint": np.ascontiguousarray(winter.astype(bf))}
    # pair pass r: lo rows = tap (r,0), hi rows = tap (r,1);
    # single pass r: tap (r,2) read from the hi (shifted) Y half
    # singles are padded to K=128 with a zero lo half: full PE row
    # utilization keeps the HAM clock governor at k=8 (K=64 streams are
    # held at half clock)
    wpair = np.zeros((4 * C, KH, O), np.float32)
    wsing = np.zeros((4 * C, KH, O), np.float32)
    for r in range(KH):
        wpair[0 : 2 * C, r] = winter[:, r * KW + 0]
        wpair[2 * C : 4 * C, r] = winter[:, r * KW + 1]
        wsing[2 * C : 4 * C, r] = winter[:, r * KW + 2]
    return {
        "wpair": np.ascontiguousarray(wpair.astype(bf)),
        "wsing": np.ascontiguousarray(wsing.astype(bf)),
    }


# ---------------------------------------------------------------- device IR


def _build_nc():
    import concourse.tile as tile
    from concourse import bacc, mybir

    f32 = mybir.dt.float32
    bf16 = mybir.dt.bfloat16
    Alu = mybir.AluOpType
    Act = mybir.ActivationFunctionType

    nc = bacc.Bacc("TRN2", target_bir_lowering=False, debug=False,
                   num_devices=NCORES)
    x_d = nc.dram_tensor("x", [IPC, C, H, W], f32, kind="ExternalInput").ap()
    if PAIR:
        wp_d = nc.dram_tensor("wpair", [4 * C, KH, O], bf16,
                              kind="ExternalInput").ap()
        ws_d = nc.dram_tensor("wsing", [4 * C, KH, O], bf16,
                              kind="ExternalInput").ap()
    else:
        wi_d = nc.dram_tensor("wint", [2 * C, K2, O], bf16,
                              kind="ExternalInput").ap()
    b_d = nc.dram_tensor("bias", [O, 1], f32, kind="ExternalInput").ap()
    o_d = nc.dram_tensor("out", [IPC, O, H, W], f32, kind="ExternalOutput").ap()

    YPART = 4 * C if PAIR else 2 * C

    with tile.TileContext(nc) as tc:
        with (
            tc.tile_pool(name="const", bufs=1) as constp,
            tc.tile_pool(name="scratch", bufs=1) as scrp,
            tc.tile_pool(name="ybuf", bufs=2) as ybufp,
            tc.tile_pool(name="psum", bufs=1, space="PSUM") as psump,
            tc.tile_pool(name="osb", bufs=2) as osbp,
        ):
            XF = scrp.tile([IPC * C, H, W], f32)
            # x row-band 0 for both images first (phi critical path)
            for i in range(IPC):
                nc.sync.dma_start(XF[i * C : (i + 1) * C, 0:17], x_d[i, :, 0:17])

            # ACT table preload + phi operands first: phi must not wait on
            # the border memsets below (gpsimd runs in issue order)
            tiny = constp.tile([IPC * C, 1], f32)
            nc.gpsimd.memset(tiny[:], 0.0)
            nc.scalar.activation(tiny[:], tiny[:], Act.Relu, bias=0.0, scale=1.0)
            negone = constp.tile([IPC * C, 1], f32)
            nc.gpsimd.memset(negone[:], -1.0)

            # PE clock ramp: dummy matmuls (results never read); use the last
            # group's PSUM banks so group 0 isn't blocked on the warm drain.
            zb = constp.tile([128, 512], bf16)
            nc.gpsimd.memset(zb[:], 0.0)
            pw = [psump.tile([O, 512], f32, name=f"ps_warm{k}",
                             tag=f"ps{6 + k}") for k in range(2)]
            for j in range(WARM):
                nc.tensor.matmul(pw[j % 2][:], zb[0:128, 0:128], zb[:],
                                 start=(j < 2), stop=(j >= WARM - 2))

            # weights + bias + rest of x
            if PAIR:
                wp_sb = constp.tile([4 * C, KH, O], bf16)
                nc.sync.dma_start(wp_sb[:], wp_d[:])
                ws_sb = constp.tile([4 * C, KH, O], bf16)
                nc.sync.dma_start(ws_sb[:], ws_d[:])
            else:
                wi_sb = constp.tile([2 * C, K2, O], bf16)
                nc.sync.dma_start(wi_sb[:], wi_d[:])
            b_sb = constp.tile([O, 1], f32)
            nc.sync.dma_start(b_sb[:], b_d[:])
            for i in range(IPC):
                nc.sync.dma_start(XF[i * C : (i + 1) * C, 17:H],
                                  x_d[i, :, 17:H])

            # coefficient planes, plane index as free dim: [2C, {c2,c4}, HP, WP]
            PLB = scrp.tile([IPC * C, 2, HP, WP], bf16)
            for g in range(2):
                bv = 1.0 if g == 0 else 0.0
                for strip in (
                    PLB[:, g, 0, :],
                    PLB[:, g, HP - 1, :],
                    PLB[:, g, 1 : HP - 1, 0],
                    PLB[:, g, 1 : HP - 1, WP - 1],
                ):
                    nc.gpsimd.memset(strip, bv)

            def phi_band(pr0, pr1):
                """c2/c4 planes for padded rows [pr0,pr1) (both images)."""
                r0, r1 = max(pr0, 1) - 1, min(pr1, HP - 1) - 1
                xf = XF[:, r0:r1]
                nc.scalar.activation(PLB[:, 1, 1 + r0 : 1 + r1, 1 : WP - 1],
                                     xf, Act.Relu, bias=negone[:], scale=2.0)
                nc.scalar.activation(PLB[:, 0, 1 + r0 : 1 + r1, 1 : WP - 1],
                                     xf, Act.Relu, bias=1.0, scale=-2.0)

            for pr0, pr1 in BANDS:
                phi_band(pr0, pr1)

            def gather(Y, i, pr0, pr1):
                """One DMA: Y[0:2C, band] = channel-interleaved c2/c4 of
                image i; PAIR adds the col+1-shifted copy in the hi half."""
                src = PLB[i * C : (i + 1) * C, :, pr0:pr1]
                nc.sync.dma_start(Y[0 : 2 * C, pr0:pr1], src)
                if PAIR:
                    nc.sync.dma_start(
                        Y[2 * C : 4 * C, pr0:pr1, 0 : WP - 1],
                        Y[0 : 2 * C, pr0:pr1, 1:WP],
                    )

            def mm_tile(Y, ps, t, first, last):
                """All tap passes for PSUM tile t (output rows RT*t..+RT)."""
                if PAIR:
                    # pair pass r: taps (r,0)+(r,1); single pass r: tap (r,2)
                    # via the shifted hi half read at +1 col (lo weights are
                    # zero) -- every matmul is K=128
                    for r in range(KH):
                        rows = slice(t * RT + r, t * RT + r + RT)
                        nc.tensor.matmul(ps[:], wp_sb[:, r, :],
                                         Y[:, rows, 0:W],
                                         start=(first and r == 0), stop=False)
                    for r in range(KH):
                        rows = slice(t * RT + r, t * RT + r + RT)
                        nc.tensor.matmul(ps[:], ws_sb[:, r, :],
                                         Y[:, rows, 1 : W + 1],
                                         start=False,
                                         stop=(last and r == KH - 1))
                else:
                    for ki in range(K2):
                        kh, kw = divmod(ki, KW)
                        rows = slice(t * RT + kh, t * RT + kh + RT)
                        nc.tensor.matmul(ps[:], wi_sb[:, ki, :],
                                         Y[:, rows, kw : kw + W],
                                         start=(first and ki == 0),
                                         stop=(last and ki == K2 - 1))

            # all gathers issue before any output DMA: the sync queue is
            # FIFO, and image 1's first matmuls must not wait behind
            # image 0's drain-gated output stores
            Ys = []
            for i in range(IPC):
                Y = ybufp.tile([YPART, HP, WP], bf16, name="Y", tag=f"Y{i}")
                for pr0, pr1 in BANDS:
                    gather(Y, i, pr0, pr1)
                Ys.append(Y)
            for i in range(IPC):
                Y = Ys[i]
                for g in range(NG):
                    osb = osbp.tile([O, GR, RT * W], f32, name="osb")
                    for j in range(GR):
                        t = g * GR + j
                        ps = psump.tile([O, RT * W], f32, name=f"ps{t}",
                                        tag=f"ps{t}")
                        mm_tile(Y, ps, t, first=True, last=True)
                        if t % 2 == 0:
                            nc.scalar.activation(osb[:, j], ps[:],
                                                 Act.Identity,
                                                 bias=b_sb[:, 0:1], scale=1.0)
                        else:
                            nc.vector.tensor_scalar(osb[:, j], ps[:],
                                                    b_sb[:, 0:1], None, Alu.add)
                    nc.sync.dma_start(
                        o_d[i, :, g * GR * RT : (g + 1) * GR * RT, :],
                        osb[:].rearrange("o g (r w) -> o (g r) w", r=RT),
                    )
    nc.compile()
    return nc


# ---------------------------------------------------------------- entry


def _prep(inputs):
    x = np.ascontiguousarray(np.asarray(inputs["x"], dtype=np.float32))
    weights = np.ascontiguousarray(np.asarray(inputs["weights"], dtype=np.float32))
    bias = np.ascontiguousarray(np.asarray(inputs["bias"], dtype=np.float32))
    positions = np.ascontiguousarray(
        np.asarray(inputs["positions"], dtype=np.float32)
    )
    return x, weights, bias, positions


def _fast_path_ok(x, positions):
    expect = np.linspace(-1.0, 1.0, P, dtype=np.float32)
    return (
        x.shape == (B, C, H, W)
        and positions.shape == (P,)
        and np.array_equal(positions, expect)
        and float(x.min()) >= 0.0
        and float(x.max()) <= 1.0
    )


def kernel(**inputs):
    x, weights, bias, positions = _prep(inputs)
    if not _fast_path_ok(x, positions):
        return _reference_np(x, weights, bias, positions)

    winter, bias_eff, ident_any = _host_weights(weights, bias)
    if ident_any:
        # identity-shortcut weights present: needs the raw-v plane; use the
        # exact fallback rather than a rarely-exercised device path
        return _reference_np(x, weights, bias, positions)

    from concourse.bass_utils import run_bass_kernel_spmd

    nc = _build_nc()
    wmap = _pack_weights(winter)
    bias2d = np.ascontiguousarray(bias_eff.reshape(O, 1))
    in_maps = [
        {"x": np.ascontiguousarray(x[i * IPC : (i + 1) * IPC]),
         "bias": bias2d, **wmap}
        for i in range(NCORES)
    ]
    res = run_bass_kernel_spmd(nc, in_maps, core_ids=list(range(NCORES)))
    out = np.concatenate([res.results[i]["out"] for i in range(NCORES)], axis=0)
    return np.ascontiguousarray(out.astype(np.float32))


# ------------------------------------------------------------ dev utilities


def _run_sim(inputs):
    """CoreSim single-core run (images 0..IPC-1) for correctness debugging."""
    from concourse.bass_interp import CoreSim

    x, weights, bias, positions = _prep(inputs)
    assert _fast_path_ok(x, positions)
    winter, bias_eff, ident_any = _host_weights(weights, bias)
    assert not ident_any
    nc = _build_nc()
    sim = CoreSim(nc)
    sim.tensor("x")[:] = x[:IPC]
    for k, v in _pack_weights(winter).items():
        sim.tensor(k)[:] = v
    sim.tensor("bias")[:] = bias_eff.reshape(O, 1)
    sim.simulate()
    return np.array(sim.tensor("out"))


# revision 17
# speedup vs baseline: 2.0992x; 1.1852x over previous
"""Trainium2 Bass kernel for CustomPositionsPiecewiseConv2d.

Math: for knots positions=[-1,-.5,0,.5,1] and x in [0,1], the active
interpolation coefficients are c2 = relu(1-2v), c4 = relu(2v-1),
c3 = 1 - c2 - c4 (exactly, everywhere incl. the zero-padding border), so
    out = C2 (x) (W2-W3) + C4 (x) (W4-W3) + sum_ck W3[o,c,k] + bias
Each plane is elementwise in v; the 3x3 im2col becomes shifted access-pattern
reads feeding PSUM-accumulated matmuls.  bf16 rounding absorbs the
isclose(v,1) mask (relu(2v-1) rounds to exactly 1.0 there); total rel err
~1e-3 vs the 2e-2 gate.

Layouts:
  PLB [2C, 2, HP, WP] bf16 - plane index is a FREE dim, so one DMA gathers
  the per-image, channel-interleaved Y = [c2/c4 x 32ch] the GEMM wants; the
  weights are row-interleaved on host to match (row 2c+g = plane-g, chan c).
  PAIR mode adds a second Y half = planes shifted one column, pairing taps
  (r,0)+(r,1) into K=128 matmuls; taps (r,2) read the shifted half at +1 col
  as K=64 singles -> 6 passes/tile instead of 9.

Pipeline: x load, phi (coeff planes), Y gather, GEMM, PSUM drain and out DMA
are all chunked into 4 row-bands per image and software-pipelined, so the PE
streams matmuls continuously from ~4us on and the tail after the last matmul
is one group's drain.

Sharding: data-parallel over batch, 2 images per core on 8 cores.
"""

import numpy as np

B, C, H, W = 16, 32, 64, 64
O, P, KH, KW = 128, 5, 3, 3
NCORES = 8
IPC = B // NCORES            # images per core
HP, WP = H + 2, W + 2        # padded image (pad=1)
RT = 8                       # output rows per PSUM tile
NT = H // RT                 # PSUM tiles per image
GR = 2                       # tiles per drain group
NG = NT // GR                # groups per image
K2 = KH * KW
ATOL = 1e-5
RTOL = 1e-5

PAIR = True                  # pair taps (r,0)+(r,1) into K=128 matmuls
WARM = 20                    # PE warmup matmuls (clock ramp)

# phi/gather row chunks (padded-row bands, group g needs bands 0..g)
BANDS = [(0, 18), (18, 34), (34, 50), (50, 66)]


# ---------------------------------------------------------------- host math


def _isclose_np(a, b):
    return np.abs(a - b) <= np.float32(ATOL) + np.float32(RTOL) * np.abs(b)


def _reference_np(x, weights, bias, positions):
    """Direct numpy port of the reference (fallback path)."""
    EPS = 1e-6
    Bn, Cn, Hn, Wn = x.shape
    On, _, Pn, KHn, KWn = weights.shape
    xp = np.pad(x, ((0, 0), (0, 0), (1, 1), (1, 1)))
    cols = [
        xp[:, :, i : i + Hn, j : j + Wn] for i in range(KHn) for j in range(KWn)
    ]
    pat = np.stack(cols, axis=2)
    v = pat.reshape(Bn, Cn, KHn * KWn, Hn * Wn).astype(np.float32)

    left, right = positions[:-1], positions[1:]
    denom = right - left
    denom = np.where(denom == 0, np.float32(EPS), denom)
    varc = (1.0 / denom).astype(np.float32)
    const = (-left * varc).astype(np.float32)

    m_first = _isclose_np(v, positions[0])
    m_last = _isclose_np(v, positions[-1])
    in_range = (~(m_first | m_last)) & (v >= positions[0]) & (v <= positions[-1])

    coeff = np.zeros(v.shape + (Pn,), np.float32)
    coeff[..., 0] += m_first.astype(np.float32)
    coeff[..., Pn - 1] += m_last.astype(np.float32)
    for p in range(Pn - 1):
        m = (in_range & (v >= positions[p]) & (v < positions[p + 1])).astype(
            np.float32
        )
        t = v * varc[p] + const[p]
        coeff[..., p] += m * (1.0 - t)
        coeff[..., p + 1] += m * t

    Wk = np.transpose(weights, (0, 1, 3, 4, 2)).reshape(On, Cn, KHn * KWn, Pn)
    ident = np.all(np.abs(Wk - 1.0) <= np.float32(ATOL + RTOL), axis=-1)
    Wk_eff = np.where(ident[..., None], np.float32(0.0), Wk)

    out = np.einsum("bcklp,ockp->bol", coeff, Wk_eff, optimize=True)
    out = out + np.einsum(
        "bckl,ock->bol", v, ident.astype(np.float32), optimize=True
    )
    out = out + bias[None, :, None]
    return out.reshape(Bn, On, Hn, Wn).astype(np.float32)


def _host_weights(weights, bias):
    """Fold c3 away and interleave rows to match the device plane layout.

    Returns (winter [2C, K2, O] f32 with row 2c+g = (W{2,4}-W3)[:,c,k].T,
    bias_eff [O] f32 = bias + sum_ck W3, ident_any)."""
    Wk = np.transpose(weights, (0, 1, 3, 4, 2)).reshape(O, C, K2, P)
    ident = np.all(np.abs(Wk - 1.0) <= np.float32(ATOL + RTOL), axis=-1)
    ident_any = bool(ident.any())
    Wk_eff = np.where(ident[..., None], np.float32(0.0), Wk)
    W3 = Wk_eff[:, :, :, 3].astype(np.float64)
    W2 = Wk_eff[:, :, :, 2].astype(np.float64) - W3   # c2 weights [O,C,K2]
    W4 = Wk_eff[:, :, :, 4].astype(np.float64) - W3   # c4 weights
    winter = np.zeros((2 * C, K2, O), np.float32)
    winter[0::2] = W2.astype(np.float32).transpose(1, 2, 0)
    winter[1::2] = W4.astype(np.float32).transpose(1, 2, 0)
    bias_eff = (bias.astype(np.float64) + W3.sum(axis=(1, 2))).astype(np.float32)
    return winter, np.ascontiguousarray(bias_eff), ident_any


def _pack_weights(winter):
    """Device weight tensors (bf16) for the chosen tap schedule."""
    import ml_dtypes

    bf = ml_dtypes.bfloat16
    if not PAIR:
        return {"wint": np.ascontiguousarray(winter.astype(bf))}
    # pair pass r: lo rows = tap (r,0), hi rows = tap (r,1);
    # single pass r: tap (r,2) read from the hi (shifted) Y half
    # singles are padded to K=128 with a zero lo half: full PE row
    # utilization keeps the HAM clock governor at k=8 (K=64 streams are
    # held at half clock)
    wpair = np.zeros((4 * C, KH, O), np.float32)
    wsing = np.zeros((4 * C, KH, O), np.float32)
    for r in range(KH):
        wpair[0 : 2 * C, r] = winter[:, r * KW + 0]
        wpair[2 * C : 4 * C, r] = winter[:, r * KW + 1]
        wsing[2 * C : 4 * C, r] = winter[:, r * KW + 2]
    return {
        "wpair": np.ascontiguousarray(wpair.astype(bf)),
        "wsing": np.ascontiguousarray(wsing.astype(bf)),
    }


# ---------------------------------------------------------------- device IR


def _build_nc():
    import concourse.tile as tile
    from concourse import bacc, mybir

    f32 = mybir.dt.float32
    bf16 = mybir.dt.bfloat16
    Alu = mybir.AluOpType
    Act = mybir.ActivationFunctionType

    nc = bacc.Bacc("TRN2", target_bir_lowering=False, debug=False,
                   num_devices=NCORES)
    x_d = nc.dram_tensor("x", [IPC, C, H, W], f32, kind="ExternalInput").ap()
    if PAIR:
        wp_d = nc.dram_tensor("wpair", [4 * C, KH, O], bf16,
                              kind="ExternalInput").ap()
        ws_d = nc.dram_tensor("wsing", [4 * C, KH, O], bf16,
                              kind="ExternalInput").ap()
    else:
        wi_d = nc.dram_tensor("wint", [2 * C, K2, O], bf16,
                              kind="ExternalInput").ap()
    b_d = nc.dram_tensor("bias", [O, 1], f32, kind="ExternalInput").ap()
    o_d = nc.dram_tensor("out", [IPC, O, H, W], bf16,
                         kind="ExternalOutput").ap()

    YPART = 4 * C if PAIR else 2 * C

    with tile.TileContext(nc) as tc:
        with (
            tc.tile_pool(name="const", bufs=1) as constp,
            tc.tile_pool(name="scratch", bufs=1) as scrp,
            tc.tile_pool(name="ybuf", bufs=2) as ybufp,
            tc.tile_pool(name="psum", bufs=1, space="PSUM") as psump,
            tc.tile_pool(name="osb", bufs=2) as osbp,
        ):
            XF = scrp.tile([IPC * C, H, W], f32)
            # x row-band 0 for both images first (phi critical path)
            for i in range(IPC):
                nc.sync.dma_start(XF[i * C : (i + 1) * C, 0:17], x_d[i, :, 0:17])

            # ACT table preload + phi operands first: phi must not wait on
            # the border memsets below (gpsimd runs in issue order)
            tiny = constp.tile([IPC * C, 1], f32)
            nc.gpsimd.memset(tiny[:], 0.0)
            nc.scalar.activation(tiny[:], tiny[:], Act.Relu, bias=0.0, scale=1.0)
            negone = constp.tile([IPC * C, 1], f32)
            nc.gpsimd.memset(negone[:], -1.0)

            # PE clock ramp: dummy matmuls (results never read); use the last
            # group's PSUM banks so group 0 isn't blocked on the warm drain.
            zb = constp.tile([128, 512], bf16)
            nc.gpsimd.memset(zb[:], 0.0)
            pw = [psump.tile([O, 512], f32, name=f"ps_warm{k}",
                             tag=f"ps{6 + k}") for k in range(2)]
            for j in range(WARM):
                nc.tensor.matmul(pw[j % 2][:], zb[0:128, 0:128], zb[:],
                                 start=(j < 2), stop=(j >= WARM - 2))

            # weights + bias + rest of x
            if PAIR:
                wp_sb = constp.tile([4 * C, KH, O], bf16)
                nc.sync.dma_start(wp_sb[:], wp_d[:])
                ws_sb = constp.tile([4 * C, KH, O], bf16)
                nc.sync.dma_start(ws_sb[:], ws_d[:])
            else:
                wi_sb = constp.tile([2 * C, K2, O], bf16)
                nc.sync.dma_start(wi_sb[:], wi_d[:])
            b_sb = constp.tile([O, 1], f32)
            nc.sync.dma_start(b_sb[:], b_d[:])
            for i in range(IPC):
                nc.sync.dma_start(XF[i * C : (i + 1) * C, 17:H],
                                  x_d[i, :, 17:H])

            # coefficient planes, plane index as free dim: [2C, {c2,c4}, HP, WP]
            PLB = scrp.tile([IPC * C, 2, HP, WP], bf16)
            for g in range(2):
                bv = 1.0 if g == 0 else 0.0
                for strip in (
                    PLB[:, g, 0, :],
                    PLB[:, g, HP - 1, :],
                    PLB[:, g, 1 : HP - 1, 0],
                    PLB[:, g, 1 : HP - 1, WP - 1],
                ):
                    nc.gpsimd.memset(strip, bv)

            def phi_band(pr0, pr1):
                """c2/c4 planes for padded rows [pr0,pr1) (both images)."""
                r0, r1 = max(pr0, 1) - 1, min(pr1, HP - 1) - 1
                xf = XF[:, r0:r1]
                nc.scalar.activation(PLB[:, 1, 1 + r0 : 1 + r1, 1 : WP - 1],
                                     xf, Act.Relu, bias=negone[:], scale=2.0)
                nc.scalar.activation(PLB[:, 0, 1 + r0 : 1 + r1, 1 : WP - 1],
                                     xf, Act.Relu, bias=1.0, scale=-2.0)

            for pr0, pr1 in BANDS:
                phi_band(pr0, pr1)

            def gather(Y, i, pr0, pr1):
                """One DMA: Y[0:2C, band] = channel-interleaved c2/c4 of
                image i; PAIR adds the col+1-shifted copy in the hi half."""
                src = PLB[i * C : (i + 1) * C, :, pr0:pr1]
                nc.sync.dma_start(Y[0 : 2 * C, pr0:pr1], src)
                if PAIR:
                    # hi half = planes shifted one col: flat views keep the
                    # runs contiguous (64 descriptors, not 64*rows); the one
                    # row-wrap garbage element lands in col WP-1, never read
                    f0, f1 = pr0 * WP, pr1 * WP
                    dst = Y[2 * C : 4 * C].rearrange("p h w -> p (h w)")
                    hsrc = PLB[i * C : (i + 1) * C].rearrange(
                        "p g h w -> p g (h w)"
                    )
                    nc.sync.dma_start(dst[:, f0 : f1 - 1],
                                      hsrc[:, :, f0 + 1 : f1])

            def mm_tile(Y, ps, t, first, last):
                """All tap passes for PSUM tile t (output rows RT*t..+RT)."""
                if PAIR:
                    # pair pass r: taps (r,0)+(r,1); single pass r: tap (r,2)
                    # via the shifted hi half read at +1 col (lo weights are
                    # zero) -- every matmul is K=128
                    for r in range(KH):
                        rows = slice(t * RT + r, t * RT + r + RT)
                        nc.tensor.matmul(ps[:], wp_sb[:, r, :],
                                         Y[:, rows, 0:W],
                                         start=(first and r == 0), stop=False)
                    for r in range(KH):
                        rows = slice(t * RT + r, t * RT + r + RT)
                        nc.tensor.matmul(ps[:], ws_sb[:, r, :],
                                         Y[:, rows, 1 : W + 1],
                                         start=False,
                                         stop=(last and r == KH - 1))
                else:
                    for ki in range(K2):
                        kh, kw = divmod(ki, KW)
                        rows = slice(t * RT + kh, t * RT + kh + RT)
                        nc.tensor.matmul(ps[:], wi_sb[:, ki, :],
                                         Y[:, rows, kw : kw + W],
                                         start=(first and ki == 0),
                                         stop=(last and ki == K2 - 1))

            # all gathers issue before any output DMA: the sync queue is
            # FIFO, and image 1's first matmuls must not wait behind
            # image 0's drain-gated output stores
            Ys = []
            for i in range(IPC):
                Y = ybufp.tile([YPART, HP, WP], bf16, name="Y", tag=f"Y{i}")
                for pr0, pr1 in BANDS:
                    gather(Y, i, pr0, pr1)
                Ys.append(Y)
            for i in range(IPC):
                Y = Ys[i]
                for g in range(NG):
                    osb = osbp.tile([O, GR, RT * W], bf16, name="osb")
                    for j in range(GR):
                        t = g * GR + j
                        ps = psump.tile([O, RT * W], f32, name=f"ps{t}",
                                        tag=f"ps{t}")
                        mm_tile(Y, ps, t, first=True, last=True)
                        if t % 2 == 0:
                            nc.scalar.activation(osb[:, j], ps[:],
                                                 Act.Identity,
                                                 bias=b_sb[:, 0:1], scale=1.0)
                        else:
                            nc.vector.tensor_scalar(osb[:, j], ps[:],
                                                    b_sb[:, 0:1], None, Alu.add)
                    nc.sync.dma_start(
                        o_d[i, :, g * GR * RT : (g + 1) * GR * RT, :],
                        osb[:].rearrange("o g (r w) -> o (g r) w", r=RT),
                    )
    nc.compile()
    return nc


# ---------------------------------------------------------------- entry


def _prep(inputs):
    x = np.ascontiguousarray(np.asarray(inputs["x"], dtype=np.float32))
    weights = np.ascontiguousarray(np.asarray(inputs["weights"], dtype=np.float32))
    bias = np.ascontiguousarray(np.asarray(inputs["bias"], dtype=np.float32))
    positions = np.ascontiguousarray(
        np.asarray(inputs["positions"], dtype=np.float32)
    )
    return x, weights, bias, positions


def _fast_path_ok(x, positions):
    expect = np.linspace(-1.0, 1.0, P, dtype=np.float32)
    return (
        x.shape == (B, C, H, W)
        and positions.shape == (P,)
        and np.array_equal(positions, expect)
        and float(x.min()) >= 0.0
        and float(x.max()) <= 1.0
    )


def kernel(**inputs):
    x, weights, bias, positions = _prep(inputs)
    if not _fast_path_ok(x, positions):
        return _reference_np(x, weights, bias, positions)

    winter, bias_eff, ident_any = _host_weights(weights, bias)
    if ident_any:
        # identity-shortcut weights present: needs the raw-v plane; use the
        # exact fallback rather than a rarely-exercised device path
        return _reference_np(x, weights, bias, positions)

    from concourse.bass_utils import run_bass_kernel_spmd

    nc = _build_nc()
    wmap = _pack_weights(winter)
    bias2d = np.ascontiguousarray(bias_eff.reshape(O, 1))
    in_maps = [
        {"x": np.ascontiguousarray(x[i * IPC : (i + 1) * IPC]),
         "bias": bias2d, **wmap}
        for i in range(NCORES)
    ]
    res = run_bass_kernel_spmd(nc, in_maps, core_ids=list(range(NCORES)))
    out = np.concatenate([res.results[i]["out"] for i in range(NCORES)], axis=0)
    return np.ascontiguousarray(out.astype(np.float32))


# ------------------------------------------------------------ dev utilities


def _run_sim(inputs):
    """CoreSim single-core run (images 0..IPC-1) for correctness debugging."""
    from concourse.bass_interp import CoreSim

    x, weights, bias, positions = _prep(inputs)
    assert _fast_path_ok(x, positions)
    winter, bias_eff, ident_any = _host_weights(weights, bias)
    assert not ident_any
    nc = _build_nc()
    sim = CoreSim(nc)
    sim.tensor("x")[:] = x[:IPC]
    for k, v in _pack_weights(winter).items():
        sim.tensor(k)[:] = v
    sim.tensor("bias")[:] = bias_eff.reshape(O, 1)
    sim.simulate()
    return np.array(sim.tensor("out"))
